# revision 36
# baseline (speedup 1.0000x reference)
"""AutoDisBucketEncoder Trainium2 kernel (8 NeuronCores, feature-sharded).

Math (per feature f, batch b):
  h = lrelu(x_aug @ w1_aug)            # bias folded via ones column
  h = lrelu(h @ (rw_l + I) + rb_l)     # x3, residual folded into weights
  z = lrelu(h @ w2 + b2)
  e = exp(z * tau)
  out = (e / sum_k e) @ emb

Layout: features sharded 32/core; each core packs 2 features per 128
partitions (block-diagonal weights), streams the full 2048 batch as the
matmul moving dim.  Softmax runs in [k, b] layout; the sum-over-k and its
broadcast back to 128 partitions are done by one ones-block matmul.  The
embedding matmul uses normalized probabilities as the stationary operand so
its PSUM output lands directly in [batch, emb] layout; four adjacent pairs
are evicted together so each output DMA writes 2KB contiguous lines.

Changes vs the first working version (227.6us -> ~185us):
  - leaky relu on the scalar engine uses Prelu (parametric_relu), which
    lives in the same activation-table set as Exp/Copy -> zero mid-kernel
    ACT_TABLE_LOADs (was 16 x 1.28us of pure scalar-engine overhead).
  - L1 matmuls run 4-pairs-concurrent via row tiling (x packed at 32j
    partition offsets; contraction is only 8 rows per pair).
  - the deferred cross-chunk work is staged so every piece enters its
    engine queue ~one h-step after its producer: t1/exp after L1, the
    sum-exp matmuls after step 1, recip/cast/mul after step 2, and the
    embedding matmuls + output copies (kept two chunks deep, so their
    inputs are always long-ready) after step 3 but before w2 -- their po
    copies then release the rotating PSUM ring slots just ahead of the
    next chunk's L1 matmuls.  This removes the head-of-line blocking
    that kept the PE semi-idle and HAM-throttled to 1.2 GHz most of the
    time.
  - the softmax-tail cast and normalize run on the otherwise-idle GPSIMD
    engine (en is not consumed for two chunks, hiding its ~2.9us/op
    latency); the last stack keeps them on the vector engine because its
    chain sits on the critical kernel tail.
  - PSUM: one pool, tag "h" ring of 3 x 2 banks shared by h/po/sum tiles
    plus a single 2-bank pz slot = exactly 8 banks; the sum tile rides
    the rotating ring so consecutive chunks' z-chains do not serialize.
  - dummy PE warmup matmuls overlap the initial constant DMAs so real
    matmuls start at the un-throttled clock.

Measured (steady state): ~181-183 us HW exec, rel err 8.7e-3; occasional
runs read ~217-220 us when the chip is thermally throttled (same code).
Of the span, ~12 us is fixed framework postamble (engine exit barrier +
final output-DMA drain) and ~130 us/engine is the scalar+vector eviction
floor imposed by f32-only PSUM on TRN2 (every PSUM->SBUF eviction runs
at 1x rate); the remaining gap is PE<->eviction handoff latency plus HAM
clock-gate oscillation, which resists further scheduling tweaks
(quadrant-packed residual matmuls and finer emb spreading were both
tried and measured slower).
"""

import sys

sys.path.insert(0, "/opt/trn_rl_repo")

import numpy as np
import ml_dtypes
from contextlib import ExitStack

BF16 = ml_dtypes.bfloat16
B, F, D, K, E = 2048, 256, 64, 8, 128
NCORES = 8
FC = F // NCORES          # 32 features per core
NPAIR = FC // 2           # 16
NSTACK = NPAIR // 4       # 4 stacks of 4 pairs
NEG = 0.01                # leaky slope
HB = B // 2               # 1024 batch half-chunk (2 PSUM banks in f32)

N_WARM_MM = 28            # dummy matmuls to lift the HAM clock gate

_compiled = None
SIM_SAFE = False  # substitute Relu for leaky so CoreSim can execute


def _h_on_scalar(step, j):
    """Engine split for the 16 h-evictions per chunk: 10 ACT / 6 DVE."""
    return not (j == 1 or (j == 3 and step % 2 == 0))


def _register_leaky_bias():
    import numpy as np
    from concourse.dve_spec import Spec, Src0, C0, C1, maxx, lower
    from concourse.dve_ops import (
        DveOp, DveOpSpec, OPS, CUSTOM_DVE_SPECS, _SUB_OPCODE_FOR_NAME,
        _CUSTOM_DVE_ROW_BASE, has_src1,
    )

    if "LEAKY_BIAS_ANT" in CUSTOM_DVE_SPECS:
        return next(o for o in OPS if o.name == "LEAKY_BIAS_ANT")
    spec = Spec(
        body=maxx(Src0 + C0, (Src0 + C0) * C1),
        reference=lambda in0, in1, s0, s1, imm2: np.maximum(
            in0 + s0, (in0 + s0) * s1
        ).astype(np.float32),
    )
    row = _CUSTOM_DVE_ROW_BASE + len(OPS)
    shas = {}
    for ver in ("v3", "v4"):
        uops = lower(spec, ver=ver)
        shas[ver] = DveOpSpec(
            name="LEAKY_BIAS_ANT", opcode=row, uops=uops, rd1_en=has_src1(spec)
        ).sha(ver)
    op = DveOp("LEAKY_BIAS_ANT", spec, subdim=False, uops_sha=shas)
    OPS.append(op)
    CUSTOM_DVE_SPECS[op.name] = spec
    _SUB_OPCODE_FOR_NAME[op.name] = row
    return op


def _build_bass():
    import concourse.bass as bass  # noqa: F401
    import concourse.mybir as mybir
    import concourse.tile as tile
    from concourse import bacc

    LEAKY_OP = _register_leaky_bias()

    dt = mybir.dt
    AF = mybir.ActivationFunctionType
    LRELU = AF.Relu if SIM_SAFE else AF.Prelu

    nc = bacc.Bacc("TRN2", target_bir_lowering=False, debug=False)

    xp = nc.dram_tensor("xp", [NSTACK, 128, B], dt.bfloat16, kind="ExternalInput").ap()
    w1p = nc.dram_tensor("w1p", [128, NSTACK * 128], dt.bfloat16, kind="ExternalInput").ap()
    rwp = nc.dram_tensor("rwp", [128, 3 * NPAIR * 128], dt.bfloat16, kind="ExternalInput").ap()
    rbp = nc.dram_tensor("rbp", [128, 3 * NPAIR], dt.float32, kind="ExternalInput").ap()
    w2p = nc.dram_tensor("w2p", [128, NPAIR * 32], dt.bfloat16, kind="ExternalInput").ap()
    b2s = nc.dram_tensor("b2s", [128, NSTACK], dt.float32, kind="ExternalInput").ap()
    taus = nc.dram_tensor("taus", [128, NSTACK], dt.float32, kind="ExternalInput").ap()
    onesbd = nc.dram_tensor("onesbd", [128, 128], dt.bfloat16, kind="ExternalInput").ap()
    embs = nc.dram_tensor("embs", [128, NSTACK * 256], dt.bfloat16, kind="ExternalInput").ap()
    out = nc.dram_tensor("out", [B, FC * E], dt.bfloat16, kind="ExternalOutput").ap()

    with tile.TileContext(nc) as tc, ExitStack() as ctx:
        const = ctx.enter_context(tc.tile_pool(name="const", bufs=1))
        xpool = ctx.enter_context(tc.tile_pool(name="xpool", bufs=4))
        hpool = ctx.enter_context(tc.tile_pool(name="hpool", bufs=12))
        tpool = ctx.enter_context(tc.tile_pool(name="tpool", bufs=3))
        epool = ctx.enter_context(tc.tile_pool(name="epool", bufs=3))
        rpool = ctx.enter_context(tc.tile_pool(name="rpool", bufs=3))
        opool = ctx.enter_context(tc.tile_pool(name="opool", bufs=6))
        # one PSUM pool: tag "h" ring of 3 x 2 banks (ph/po/sum), tag "pz"
        # 1 x 2 banks -> exactly 8 banks
        ps = ctx.enter_context(tc.tile_pool(name="ps", bufs=3, space="PSUM"))

        # ---- constants into SBUF; order so L1 deps land first and the PE
        # warms up on dummy matmuls while the bulk DMAs run ----
        ones_sb = const.tile([128, 128], dt.bfloat16)
        nc.sync.dma_start(out=ones_sb, in_=onesbd)
        w1_sb = const.tile([128, NSTACK * 128], dt.bfloat16)
        nc.sync.dma_start(out=w1_sb, in_=w1p)
        rb_sb = const.tile([128, 3 * NPAIR], dt.float32)
        nc.sync.dma_start(out=rb_sb, in_=rbp)

        # prefetch the first two chunks' x slices ahead of the bulk constants
        x_first = xpool.tile([128, HB], dt.bfloat16, tag="x", name="x_s0_c0")
        nc.sync.dma_start(out=x_first, in_=xp[0][:, 0:HB])
        x_second = xpool.tile([128, HB], dt.bfloat16, tag="x", name="x_s0_c1")
        nc.sync.dma_start(out=x_second, in_=xp[0][:, HB:B])

        # PE warmup: harmless matmuls that only depend on ones_sb
        warm_ps = ps.tile([128, 128], dt.float32, tag="h", name="warm")
        for _ in range(N_WARM_MM):
            nc.tensor.matmul(warm_ps, ones_sb, ones_sb, start=True, stop=True)

        # rw arrives split: the first chunk's four pair-blocks of each
        # layer first, so chunk (0,0)'s residual steps never wait on the
        # 1.5MB bulk transfer
        rw_sb = const.tile([128, 3 * NPAIR * 128], dt.bfloat16)
        for l in range(3):
            base = l * NPAIR * 128
            nc.sync.dma_start(
                out=rw_sb[:, base : base + 512],
                in_=rwp[:, base : base + 512],
            )
        for l in range(3):
            base = l * NPAIR * 128
            nc.sync.dma_start(
                out=rw_sb[:, base + 512 : base + NPAIR * 128],
                in_=rwp[:, base + 512 : base + NPAIR * 128],
            )
        w2_sb = const.tile([128, NPAIR * 32], dt.bfloat16)
        nc.sync.dma_start(out=w2_sb, in_=w2p)
        b2_sb = const.tile([128, NSTACK], dt.float32)
        nc.sync.dma_start(out=b2_sb, in_=b2s)
        tau_sb = const.tile([128, NSTACK], dt.float32)
        nc.sync.dma_start(out=tau_sb, in_=taus)
        emb_sb = const.tile([128, NSTACK * 256], dt.bfloat16)
        nc.sync.dma_start(out=emb_sb, in_=embs)

        # out[b, fc*E] viewed as [qb(4), s(4), p(128), i(4), j(4), e(256)]
        out_r = out.rearrange("(qb i p) (s j e) -> qb s p i j e", p=128, i=4, j=4, e=256)

        def evict_h(step, j, h, ph, rb_ap):
            """psum -> sbuf bf16 with (optional bias add and) leaky relu."""
            if _h_on_scalar(step, j):
                if rb_ap is None:
                    nc.scalar.activation(h, ph, LRELU, alpha=NEG)
                else:
                    nc.scalar.activation(h, ph, LRELU, bias=rb_ap, alpha=NEG)
            else:
                nc.vector._custom_dve(
                    LEAKY_OP,
                    out=h,
                    in0=ph,
                    s0=0.0 if rb_ap is None else rb_ap,
                    s1=NEG,
                )

        # Deferred work from the previous chunk, staged so that each piece
        # enters its engine queue ~one h-step after its producer was
        # enqueued (avoids head-of-line blocking, keeps the PE dense):
        #   after L1     : t1+exp (scalar)        <- needs pz (ready at entry)
        #   after step 1 : sumexp matmuls (PE)    <- needs exp
        #   after step 2 : recip/cast/mul (DVE)   <- needs sum matmuls
        #   after w2     : emb matmuls + po copies <- needs mul
        pending_t1exp = []
        pending_summ = []
        pending_tail = []
        pending_emb = []

        def emit(lst, n=99):
            for _ in range(min(n, len(lst))):
                lst.pop(0)()

        for s in range(NSTACK):
            e_sb = epool.tile([128, B], dt.bfloat16, tag="e", name=f"e{s}")
            en_sb = epool.tile([128, B], dt.bfloat16, tag="en", name=f"en{s}")
            for c in range(2):
                # ---- x for all 4 pairs of this chunk in one [128, HB] tile
                if s == 0 and c == 0:
                    x_sb = x_first
                elif s == 0 and c == 1:
                    x_sb = x_second
                else:
                    x_sb = xpool.tile([128, HB], dt.bfloat16, tag="x", name=f"x{s}_{c}")
                    nc.sync.dma_start(out=x_sb, in_=xp[s][:, c * HB : (c + 1) * HB])
                hs = [None] * 4
                # ---- L1: contraction is 8 rows per pair -> 4 concurrent
                # row-tiled matmuls at 32j offsets
                for j in range(4):
                    ph = ps.tile([128, HB], dt.float32, tag="h", name=f"ph{s}_{j}_{c}_0")
                    w_l1 = w1_sb[32 * j : 32 * j + 8, s * 128 : (s + 1) * 128]
                    for q in range(2):
                        nc.tensor.matmul(
                            ph[:, q * 512 : (q + 1) * 512],
                            w_l1,
                            x_sb[32 * j : 32 * j + 8, q * 512 : (q + 1) * 512],
                            start=True,
                            stop=True,
                            tile_position=(32 * j, 0),
                        )
                    h2 = hpool.tile([128, HB], dt.bfloat16, tag="h", name=f"h{s}_{j}_{c}_0")
                    evict_h(0, j, h2, ph, None)
                    hs[j] = h2
                emit(pending_t1exp)
                # ---- residual layers ----
                for step in range(1, 4):
                    l = step - 1
                    for j in range(4):
                        p = 4 * s + j
                        ph = ps.tile(
                            [128, HB], dt.float32, tag="h", name=f"ph{p}_{c}_{step}"
                        )
                        wsl = rw_sb[
                            :, (l * NPAIR + p) * 128 : (l * NPAIR + p + 1) * 128
                        ]
                        rb_ap = rb_sb[:, l * NPAIR + p : l * NPAIR + p + 1]
                        for q in range(2):
                            nc.tensor.matmul(
                                ph[:, q * 512 : (q + 1) * 512],
                                wsl,
                                hs[j][:, q * 512 : (q + 1) * 512],
                                start=True,
                                stop=True,
                            )
                        h2 = hpool.tile(
                            [128, HB], dt.bfloat16, tag="h", name=f"h{p}_{c}_{step}"
                        )
                        evict_h(step, j, h2, ph, rb_ap)
                        hs[j] = h2
                    if step == 1:
                        emit(pending_summ)
                    elif step == 2:
                        emit(pending_tail)
                    else:
                        # chunk c-2's emb groups (inputs long-ready): before
                        # w2, so their po copies release the psum ring slots
                        # ahead of the next chunk's L1 matmuls
                        emit(pending_emb, max(0, len(pending_emb) - 8))
                pz = ps.tile([128, HB], dt.float32, tag="pz", bufs=1, name=f"pz{s}_{c}")
                for j in range(4):
                    p = 4 * s + j
                    for q in range(2):
                        nc.tensor.matmul(
                            pz[32 * j : 32 * j + 32, q * 512 : (q + 1) * 512],
                            w2_sb[:, p * 32 : (p + 1) * 32],
                            hs[j][:, q * 512 : (q + 1) * 512],
                            start=True,
                            stop=True,
                            tile_position=(0, 32 * j),
                        )


                def make_z(s_, c_, pz_ref, e_ref, en_ref):
                    ev = e_ref[:, c_ * HB : (c_ + 1) * HB]

                    def t1exp():
                        t1 = tpool.tile(
                            [128, HB], dt.float32, tag="zt", name=f"t1_{s_}_{c_}"
                        )
                        nc.scalar.activation(
                            t1, pz_ref, LRELU,
                            bias=b2_sb[:, s_ : s_ + 1], alpha=NEG,
                        )
                        nc.scalar.activation(
                            ev, t1, AF.Exp, scale=tau_sb[:, s_ : s_ + 1]
                        )

                        def summ():
                            ps_sum = ps.tile(
                                [128, HB], dt.float32, tag="h", name=f"psum{s_}_{c_}"
                            )
                            for q in range(2):
                                nc.tensor.matmul(
                                    ps_sum[:, q * 512 : (q + 1) * 512],
                                    ones_sb,
                                    ev[:, q * 512 : (q + 1) * 512],
                                    start=True,
                                    stop=True,
                                )

                            def tail():
                                rcf = rpool.tile(
                                    [128, HB], dt.float32, tag="rcf",
                                    name=f"rcf{s_}_{c_}"
                                )
                                nc.vector.reciprocal_approx_fast(out=rcf, in_=ps_sum)
                                rc = rpool.tile(
                                    [128, HB], dt.bfloat16, tag="rc",
                                    name=f"rc{s_}_{c_}"
                                )
                                # cast + normalize run on the otherwise-idle
                                # GPSIMD engine (~2.9us/op vs 0.6-0.7 on DVE);
                                # mid-kernel the 2-chunk emb deferral hides
                                # that latency, but the last stack's chain is
                                # on the critical tail, so keep it on DVE
                                eng = nc.vector if s_ == NSTACK - 1 else nc.gpsimd
                                eng.tensor_copy(rc, rcf)
                                eng.tensor_mul(
                                    en_ref[:, c_ * HB : (c_ + 1) * HB], ev, rc
                                )

                            pending_tail.append(tail)

                        pending_summ.append(summ)

                    return t1exp

                pending_t1exp.append(make_z(s, c, pz, e_sb, en_sb))

                def make_group(s_, qb_, j_, en_ref, ob_ref):
                    def emit_group():
                        po = ps.tile(
                            [128, 4, 256],
                            dt.float32,
                            tag="h",
                            name=f"po{s_}_{qb_}_{j_}",
                        )
                        for i in range(4):
                            bc2 = qb_ * 4 + i
                            nc.tensor.matmul(
                                po[:, i, :],
                                en_ref[
                                    32 * j_ : 32 * j_ + 16, bc2 * 128 : (bc2 + 1) * 128
                                ],
                                emb_sb[32 * j_ : 32 * j_ + 16, s_ * 256 : (s_ + 1) * 256],
                                start=True,
                                stop=True,
                                tile_position=(32 * j_, 0),
                            )
                        on_act = j_ == 3
                        if on_act:
                            nc.scalar.copy(ob_ref[:, :, j_, :], po)
                        else:
                            nc.vector.tensor_copy(ob_ref[:, :, j_, :], po)
                        if s_ == NSTACK - 1:
                            # last stack: halve the output DMAs so the final
                            # wire drain (which the postamble waits on)
                            # starts as soon as the first half is assembled
                            if j_ == 1:
                                nc.sync.dma_start(
                                    out=out_r[qb_, s_][:, :, 0:2, :],
                                    in_=ob_ref[:, :, 0:2, :],
                                )
                            elif j_ == 3:
                                nc.sync.dma_start(
                                    out=out_r[qb_, s_][:, :, 2:4, :],
                                    in_=ob_ref[:, :, 2:4, :],
                                )
                        elif j_ == 3:
                            nc.sync.dma_start(out=out_r[qb_, s_], in_=ob_ref)

                    return emit_group

                for qb in (2 * c, 2 * c + 1):
                    ob = opool.tile(
                        [128, 4, 4, 256], dt.bfloat16, tag="o", name=f"ob{s}_{qb}"
                    )
                    for j in range(4):
                        pending_emb.append(make_group(s, qb, j, en_sb, ob))
        emit(pending_t1exp)
        emit(pending_summ)
        emit(pending_tail)
        emit(pending_emb)

    nc.compile()
    return nc


def _host_pack(inputs):
    """Pack full f32 inputs into per-core bf16 device arrays."""
    x = np.ascontiguousarray(inputs["x"], dtype=np.float32)
    w1 = np.asarray(inputs["w1"], dtype=np.float32)
    b1 = np.asarray(inputs["b1"], dtype=np.float32)
    w2 = np.asarray(inputs["w2"], dtype=np.float32)
    b2 = np.asarray(inputs["b2"], dtype=np.float32)
    tau = np.asarray(inputs["tau"], dtype=np.float32)
    emb = np.asarray(inputs["emb"], dtype=np.float32)
    rws = [np.asarray(inputs[f"rw{l}"], dtype=np.float32) for l in range(3)]
    rbs = [np.asarray(inputs[f"rb{l}"], dtype=np.float32) for l in range(3)]

    eye = np.eye(D, dtype=np.float32)
    xT = np.concatenate([x, np.ones((B, F, 1), np.float32)], axis=2)
    xT = np.ascontiguousarray(xT.transpose(1, 2, 0))  # [F, 4, B]
    w1a = np.concatenate([w1, b1[:, None, :]], axis=1)  # [F, 4, D]

    in_maps = []
    for cidx in range(NCORES):
        f0 = cidx * FC
        xpk = np.zeros((NSTACK, 128, B), BF16)
        w1k = np.zeros((128, NSTACK, 128), BF16)
        rwk = np.zeros((128, 3, NPAIR, 128), BF16)
        rbk = np.zeros((128, 3, NPAIR), np.float32)
        w2k = np.zeros((128, NPAIR, 32), BF16)
        b2k = np.zeros((128, NSTACK), np.float32)
        tauk = np.zeros((128, NSTACK), np.float32)
        # garbage partitions keep tau=0 so exp(0)=1 stays finite
        embk = np.zeros((128, NSTACK, 256), BF16)
        for pr in range(NPAIR):
            fa, fb = f0 + 2 * pr, f0 + 2 * pr + 1
            s, jj = pr // 4, pr % 4
            xpk[s, 32 * jj : 32 * jj + 4] = xT[fa]
            xpk[s, 32 * jj + 4 : 32 * jj + 8] = xT[fb]
            w1k[32 * jj : 32 * jj + 4, s, 0:64] = w1a[fa]
            w1k[32 * jj + 4 : 32 * jj + 8, s, 64:128] = w1a[fb]
            for l in range(3):
                rwk[0:64, l, pr, 0:64] = rws[l][fa] + eye
                rwk[64:128, l, pr, 64:128] = rws[l][fb] + eye
                rbk[0:64, l, pr] = rbs[l][fa]
                rbk[64:128, l, pr] = rbs[l][fb]
            w2k[0:64, pr, 0:8] = w2[fa]
            w2k[64:128, pr, 8:16] = w2[fb]
            for fi, ff in ((0, fa), (1, fb)):
                rows = slice(32 * jj + 8 * fi, 32 * jj + 8 * fi + 8)
                b2k[rows, s] = b2[ff]
                tauk[rows, s] = tau[ff]
                embk[rows, s, 128 * fi : 128 * fi + 128] = emb[ff]
        # sum-over-k stationary with broadcast to all 128 rows; garbage
        # partitions duplicate the pair's second feature so values stay sane.
        ob = np.zeros((128, 128), BF16)
        for jj in range(4):
            for g in range(4):
                src = 32 * jj + 8 * min(g, 1)
                ob[src : src + 8, 32 * jj + 8 * g : 32 * jj + 8 * g + 8] = 1
        m = {
            "xp": xpk,
            "w1p": w1k.reshape(128, NSTACK * 128),
            "rwp": rwk.reshape(128, 3 * NPAIR * 128),
            "rbp": rbk.reshape(128, 3 * NPAIR),
            "w2p": w2k.reshape(128, NPAIR * 32),
            "b2s": b2k,
            "taus": tauk,
            "embs": embk.reshape(128, NSTACK * 256),
            "onesbd": ob,
        }
        in_maps.append(m)
    return in_maps


def _get_compiled():
    global _compiled
    if _compiled is None:
        _compiled = _build_bass()
    return _compiled


def run_on_hw(in_maps, trace=False):
    from concourse import bass_utils

    nc = _get_compiled()
    res = bass_utils.run_bass_kernel_spmd(
        nc, in_maps, core_ids=list(range(NCORES)), trace=trace
    )
    return res


def kernel(**inputs):
    in_maps = _host_pack(inputs)
    res = run_on_hw(in_maps, trace=False)
    outs = [np.asarray(res.results[c]["out"], dtype=np.float32) for c in range(NCORES)]
    return np.concatenate(outs, axis=1)
